# revision 6
# baseline (speedup 1.0000x reference)
"""Trainium2 Bass kernel for the NeuralODE problem.

Math (matching reference.py):
    20 Euler steps (10 segments x 2 steps, uniform dt => step size hi = 0.05):
        z_{i+1} = z_i + hi * ( tanh(z_i @ W1 + b1 + t_i*wt) @ W2 + b2 )

Device-side reformulation (per core, batch shard B=64):
    - Fold hi into W2:  W2' = hi * W2, c = hi * b2.
    - Keep the "state without accumulated c":  z'_i = z_i - i*c, so
        z'_{i+1} = z'_i + tanh(z'_i @ W1 + bias_i) @ W2'
      with bias_i = b1 + t_i*wt + i*(c @ W1)   (precomputed on host).
      Final output: z_20 = z'_20 + 20*c       (added on host).
    - State kept transposed (d-major) as zT[p, 64k+b] = z'[b, 128k+p] so it can
      be the stationary (lhsT) operand of orientation-B matmuls.
    - Matmuls run in bf16 (1 cyc/col on the PE vs 4 for fp32); the Euler state
      itself stays fp32 in SBUF (bf16 state would absorb the small 0.05*f
      increments), with a bf16 shadow copy produced each step for the PE.
    - Both matmuls stream the (SBUF-resident) weights as the moving operand with
      N=512 chunks; the 64-wide batch stationary only fills half the PE columns,
      so two chunks run concurrently via tile_position col-tiling (0,0)/(0,64).
    - The PE pair rate is LDWEIGHTS-bound at 1 load per matmul (2x131ns >
      512/2.4GHz), so:
        mm1 runs k-major: each z k-tile stationary serves all 4 HID chunks
          (2 concurrent pairs); the redundant 2 LDWEIGHTS are stripped from the
          BIR post-compile (hardware keeps weights loaded across matmuls).
        mm2 alternates k-tiles between the two column positions (even k at
          cols 0-63, odd at 64-127), so each k-tile is loaded once; the two
          per-position partial sums land in different PSUM rows and are folded
          with one DVE add afterwards.
    - The per-step bias enters PSUM first through a K=1 ones-vector matmul.
    - Layout flips (batch-major PSUM result -> d/hid-major stationary for the
      next matmul) are PE transpose-mode matmuls against identity, pipelined in
      128-column blocks: tanh -> transpose -> copy -> dependent matmuls, so the
      serial chains at step boundaries stay short.

Sharding: pure data-parallel over batch (512 -> 8 x 64); weights replicated.
"""

import numpy as np
import ml_dtypes

BS, D, HID = 512, 1024, 2048
NCORES = 8
B = BS // NCORES  # 64
NSTEP = 20
KD = D // 128  # 8 k-tiles for the D contraction
KH = HID // 128  # 16 k-tiles for the HID contraction
F32 = np.float32
BF16 = ml_dtypes.bfloat16

MM_DTYPE = "bfloat16"  # matmul dtype: "bfloat16" (1 cyc/col) or "float32" (4 cyc/col)


def _strip_redundant_ldweights(nc):
    """Remove InstLdweights that reload the stationary already resident at the
    same PE column position (no intervening conflicting load), so consecutive
    matmuls sharing a stationary pay for one LDWEIGHTS. Only sync-free loads
    are stripped; any load carrying waits/updates is kept."""
    import concourse.mybir as mybir

    n_strip = 0
    for func in nc.m.functions:
        for blk in func.blocks:
            loaded = {}  # tile_position -> content key
            keep = []
            for inst in blk.instructions:
                if isinstance(inst, mybir.InstLdweights):
                    ap = inst.ins[0]
                    key = (
                        ap.memref,
                        ap.offset,
                        str(ap.ap),
                        str(getattr(inst, "is_transpose", None)),
                        str(getattr(inst, "tile_size", None)),
                    )
                    tp = tuple(getattr(inst, "tile_position", None) or (0, 0))
                    si = inst.sync_info
                    clean = si is None or (not si.on_wait and not si.on_update)
                    tsz = getattr(inst, "tile_size", None)
                    wide = tsz is None or tsz[1] > 64  # covers both col halves
                    if clean and not wide and loaded.get(tp) == key:
                        n_strip += 1
                        continue
                    if wide:
                        loaded.clear()
                    loaded[tp] = key
                keep.append(inst)
            blk.instructions[:] = keep
    return n_strip


def _build_program(mm_dtype=MM_DTYPE):
    import concourse.mybir as mybir
    from concourse import bacc
    from concourse.tile import TileContext

    nc = bacc.Bacc()
    f32 = mybir.dt.float32
    mmdt = getattr(mybir.dt, mm_dtype)
    lowp = mmdt != f32
    TANH = mybir.ActivationFunctionType.Tanh

    zt_in = nc.dram_tensor("zt_in", [128, KD * B], f32, kind="ExternalInput")
    w1_d = nc.dram_tensor("w1", [128, KD * HID], mmdt, kind="ExternalInput")
    w2_d = nc.dram_tensor("w2", [128, KH * D], mmdt, kind="ExternalInput")
    biases_d = nc.dram_tensor("biases", [NSTEP, HID], mmdt, kind="ExternalInput")
    ident_d = nc.dram_tensor("ident", [128, 128], mmdt, kind="ExternalInput")
    ones_d = nc.dram_tensor("ones", [1, B], mmdt, kind="ExternalInput")
    zt_out = nc.dram_tensor("zt_out", [128, KD * B], f32, kind="ExternalOutput")

    def ublk(t):  # [128, 512] laid out (h, u, c) -> [p, h, u, c]
        return t.rearrange("p (h u c) -> p h u c", h=2, u=4)

    def tblk(t):  # transpose-psum [128, 512] laid out (u, h, c) -> [p, h, u, c]
        return t.rearrange("p (u h c) -> p h u c", u=4, h=2)

    with (
        TileContext(nc) as tc,
        tc.tile_pool(name="const", bufs=1) as cpool,
        tc.tile_pool(name="weights", bufs=1) as wpool,
        tc.tile_pool(name="state", bufs=1) as spool,
        tc.tile_pool(name="work", bufs=2) as hpool,
        tc.tile_pool(name="bias", bufs=2) as bpool,
        tc.tile_pool(name="psumh", bufs=1, space="PSUM") as ph_pool,
        tc.tile_pool(name="psumt", bufs=1, space="PSUM") as pt_pool,
        tc.tile_pool(name="psumf", bufs=1, space="PSUM") as pf_pool,
    ):
        ident_sb = cpool.tile([128, 128], mmdt, tag="ident")
        nc.sync.dma_start(ident_sb[:], ident_d[:])
        ones_sb = cpool.tile([1, B], mmdt, tag="ones")
        nc.sync.dma_start(ones_sb[:], ones_d[:])

        zt = spool.tile([128, KD * B], f32, tag="zt")  # fp32 z'_T state [128, 512]
        nc.sync.dma_start(zt[:], zt_in[:])
        if lowp:
            zmm = spool.tile([128, KD * B], mmdt, tag="zmm")  # bf16 shadow for PE
            nc.vector.tensor_copy(zmm[:], zt[:])
        else:
            zmm = zt
        hT = spool.tile([128, KH * B], mmdt, tag="hT")  # tanh'd h, hid-major [128,1024]

        # per-k weight tiles so step-0 matmuls can start as soon as their
        # own k-slice has landed (whole-tensor deps would stall ~25us)
        w1t = []
        for k in range(KD):
            w = wpool.tile([128, HID], mmdt, tag=f"w1_{k}")
            nc.sync.dma_start(w[:], w1_d[:, k * HID : (k + 1) * HID])
            w1t.append(w)
        w2t = []
        for k in range(KH):
            w = wpool.tile([128, D], mmdt, tag=f"w2_{k}")
            nc.sync.dma_start(w[:], w2_d[:, k * D : (k + 1) * D])
            w2t.append(w)

        # mm1 k-order: u-major so each k needs only the state u-block that the
        # boundary pipeline has produced most recently (k = 4h + u).
        K_ORDER = [0, 4, 1, 5, 2, 6, 3, 7]

        for i in range(NSTEP):
            bias_sb = bpool.tile([1, HID], mmdt, tag="bias")
            nc.sync.dma_start(bias_sb[:], biases_d[i : i + 1, :])

            # ---- mm1: h_pre = z @ W1 + bias_i ----
            # chunk c of 512 HID cols lives in psum bank c//2, rows 64*(c%2).
            # k-major: one stationary (z k-tile, both col positions) serves all
            # 4 chunks; redundant LDWEIGHTS are stripped post-compile.
            ph_a = ph_pool.tile([128, 512], f32, tag="phA")
            ph_b = ph_pool.tile([128, 512], f32, tag="phB")
            phs = [ph_a, ph_b]
            for c in range(4):
                nc.tensor.matmul(
                    phs[c // 2][64 * (c % 2) : 64 * (c % 2) + 64, :],
                    ones_sb[:1, :],
                    bias_sb[:1, 512 * c : 512 * c + 512],
                    start=True,
                    stop=False,
                    tile_position=(0, 64 * (c % 2)),
                )
            for ki, k in enumerate(K_ORDER):
                last = ki == KD - 1
                for c in range(4):
                    nc.tensor.matmul(
                        phs[c // 2][64 * (c % 2) : 64 * (c % 2) + 64, :],
                        zmm[:, B * k : B * k + B],
                        w1t[k][:, 512 * c : 512 * c + 512],
                        start=False,
                        stop=last,
                        tile_position=(0, 64 * (c % 2)),
                    )

            # ---- tanh -> transpose -> hT copy -> mm2, pipelined per (g, u) ----
            # mm2 k-tiles kA=8g+u (cols 0-63) / kB=8g+4+u (cols 64-127) use the
            # hT block written by copy (g, u); per-position partial sums are
            # folded after the loop.  pf chunk c: rows 0-63 = sum over kA's,
            # rows 64-127 = sum over kB's.
            h_bms = []
            pts = []
            pf_a = pf_pool.tile([128, 512], f32, tag="pfA")
            pf_b = pf_pool.tile([128, 512], f32, tag="pfB")
            pfs = [pf_a, pf_b]
            for g in range(2):
                h_bm = hpool.tile([128, 512], mmdt, tag=f"h_bm{g}")
                h_bms.append(h_bm)
                pt = pt_pool.tile([128, 512], mmdt, tag=f"pt{g}")
                pts.append(pt)
            for g in range(2):
                for u in range(4):
                    sl = slice(128 * u, 128 * u + 128)
                    nc.scalar.activation(h_bms[g][:, sl], phs[g][:, sl], TANH)
                    nc.tensor.matmul(
                        pts[g][:, sl],
                        h_bms[g][:, sl],
                        ident_sb[:],
                        is_transpose=True,
                        start=True,
                        stop=True,
                    )
                    nc.vector.tensor_copy(
                        ublk(hT[:, 512 * g : 512 * g + 512])[:, :, u : u + 1, :],
                        tblk(pts[g][:])[:, :, u : u + 1, :],
                    )
                    kA, kB = 8 * g + u, 8 * g + 4 + u
                    first, last = (g == 0 and u == 0), (g == 1 and u == 3)
                    for c in range(2):
                        nc.tensor.matmul(
                            pfs[c][0:64, :],
                            hT[:, B * kA : B * kA + B],
                            w2t[kA][:, 512 * c : 512 * c + 512],
                            start=first,
                            stop=last,
                            tile_position=(0, 0),
                        )
                        nc.tensor.matmul(
                            pfs[c][64:128, :],
                            hT[:, B * kB : B * kB + B],
                            w2t[kB][:, 512 * c : 512 * c + 512],
                            start=first,
                            stop=last,
                            tile_position=(0, 64),
                        )

            # ---- fold mm2 halves, transpose f to d-major, update state ----
            # All split per 128-col u-block so next step's mm1 (k = 4h + u,
            # issued u-major) starts as soon as block u0 is through.
            f_bm = hpool.tile([128, 512], mmdt, tag="f_bm")
            f_tmp = hpool.tile([128, 512], mmdt, tag="f_tmp")
            pt2 = pt_pool.tile([128, 512], mmdt, tag="pt2")
            for u in range(4):
                sl = slice(128 * u, 128 * u + 128)
                # one PSUM read per instruction: copy the odd-position partial
                # out first, then add it to the even-position partial.
                for c in range(2):
                    nc.vector.tensor_copy(
                        f_tmp[64 * c : 64 * c + 64, sl], pfs[c][64:128, sl]
                    )
                for c in range(2):
                    nc.vector.tensor_add(
                        f_bm[64 * c : 64 * c + 64, sl],
                        pfs[c][0:64, sl],
                        f_tmp[64 * c : 64 * c + 64, sl],
                    )
                nc.tensor.matmul(
                    pt2[:, sl],
                    f_bm[:, sl],
                    ident_sb[:],
                    is_transpose=True,
                    start=True,
                    stop=True,
                )
                if lowp:
                    nc.vector.tensor_add(
                        ublk(zmm[:])[:, :, u : u + 1, :],
                        ublk(zt[:])[:, :, u : u + 1, :],
                        tblk(pt2[:])[:, :, u : u + 1, :],
                    )
            # fp32 master state update; off the critical path (next mm1 only
            # needs zmm).  Ordered after the zmm reads of zt (DVE is FIFO).
            nc.vector.tensor_add(ublk(zt[:]), ublk(zt[:]), tblk(pt2[:]))

        nc.sync.dma_start(zt_out[:], zt[:])

    nc.compile()
    stripped = _strip_redundant_ldweights(nc)
    assert stripped >= NSTEP * 2 * (KD - 1), f"only stripped {stripped} ldweights"
    return nc


def _pack_zT(shard):  # [B, D] -> [128, KD*B]
    return np.ascontiguousarray(
        shard.T.reshape(KD, 128, B).transpose(1, 0, 2).reshape(128, KD * B)
    )


def _unpack_zT(zt):  # [128, KD*B] -> [B, D]
    return zt.reshape(128, KD, B).transpose(1, 0, 2).reshape(D, B).T


def _host_inputs(z0, t, W1, b1, wt, W2, b2, npdt):
    t = np.asarray(t, F32)
    t0s, t1s = t[:-1], t[1:]
    h_seg = (t1s - t0s) / 2.0  # N_STEPS_PER_SEG = 2
    step_ts = (t0s[:, None] + h_seg[:, None] * np.arange(2, dtype=F32)[None, :]).reshape(
        -1
    )
    step_hs = np.repeat(h_seg, 2)
    assert np.allclose(step_hs, step_hs[0]), "non-uniform Euler steps unsupported"
    scale = F32(step_hs[0])

    c = (scale * np.asarray(b2, F32)).astype(F32)  # [D]
    cW1 = (c.astype(np.float64) @ np.asarray(W1, np.float64)).astype(F32)  # [HID]
    biases = np.stack(
        [
            (np.asarray(b1, F32) + step_ts[i] * np.asarray(wt, F32) + i * cW1).astype(
                F32
            )
            for i in range(NSTEP)
        ]
    ).astype(npdt)  # [NSTEP, HID]

    w1p = np.ascontiguousarray(
        np.asarray(W1, F32).reshape(KD, 128, HID).transpose(1, 0, 2).reshape(128, KD * HID)
    ).astype(npdt)
    w2p = np.ascontiguousarray(
        (scale * np.asarray(W2, F32))
        .astype(F32)
        .reshape(KH, 128, D)
        .transpose(1, 0, 2)
        .reshape(128, KH * D)
    ).astype(npdt)
    ident = np.eye(128, dtype=npdt)
    ones = np.ones((1, B), npdt)
    return biases, w1p, w2p, ident, ones, c


def _make_in_maps(z0, t, W1, b1, wt, W2, b2, npdt):
    z0 = np.asarray(z0, F32)
    biases, w1p, w2p, ident, ones, c = _host_inputs(z0, t, W1, b1, wt, W2, b2, npdt)
    in_maps = []
    for core in range(NCORES):
        shard = z0[core * B : (core + 1) * B]
        in_maps.append(
            {
                "zt_in": _pack_zT(shard),
                "w1": w1p,
                "w2": w2p,
                "biases": biases,
                "ident": ident,
                "ones": ones,
            }
        )
    return in_maps, c


def run(z0, t, W1, b1, wt, W2, b2, trace=False, mm_dtype=MM_DTYPE):
    from concourse.bass_utils import run_bass_kernel_spmd

    npdt = F32 if mm_dtype == "float32" else BF16
    in_maps, c = _make_in_maps(z0, t, W1, b1, wt, W2, b2, npdt)
    nc = _build_program(mm_dtype=mm_dtype)
    res = run_bass_kernel_spmd(nc, in_maps, core_ids=list(range(NCORES)), trace=trace)

    outs = []
    for core in range(NCORES):
        z_shard = _unpack_zT(np.asarray(res.results[core]["zt_out"], F32))
        outs.append(z_shard)
    out = np.concatenate(outs, axis=0).astype(F32)
    out = out + (NSTEP * c)[None, :].astype(F32)
    return out.astype(F32), res


def kernel(z0, t, W1, b1, wt, W2, b2):
    out, _ = run(z0, t, W1, b1, wt, W2, b2, trace=False)
    return out
